# revision 67
# baseline (speedup 1.0000x reference)
"""Trainium2 Bass kernel for nn_CLNF_54769422959177 (v2).

Computes (dp, dw): dp = vf(p) (4-layer softplus+LN VectorField forward) and
dw = -vjp(vf, p)(w), data-parallel over 8 NeuronCores.

Design (1179 us v1 baseline -> 528 us):
- single pre-placed natural_log_exp act-table load (v1 had 443 loads/568us)
- all bulk DVE operands fp16 in SBUF so TensorScalar ops hit the 4x perf
  mode; per-row stats applied via per-block tensor_scalar with dual f32
  scalar pointers (the only per-row-scalar primitive with fast modes)
- LN mean/var via per-block bn_stats (one DVE pass); even/odd halves
  combined with tiny Pool ops; the /H fold rides the ACT Ln scale;
  rstd = exp(-0.5*ln(var+eps)) keeps everything in act-table set 6
- sigmoid saved in fwd (sig = 1 - exp(-A)); backward softplus+LN chain is
  three fp16 TTs + per-block TS; the rstd multiply is folded out of the
  whole backward chain and applied once in the dw output copy
  (F = prod_i rstd_i per row)
- inputs host-pretransposed to fp16 [128 feat, b, 128 rows] so first-layer
  matmul stationaries come straight from DMA; outputs written fp16 and
  converted on host
- GBLK=4, three forward + three backward generator streams interleaved in
  lockstep chunks with per-stream PSUM pools (3 zpf + 3 zpb + 2 tp banks);
  engine assignment balanced: ACT {Exp, Ln, exp(-A), stat Ln/Exp, PSUM
  copies}, DVE {bn_stats, m2/xh/u TS, TTs, transpose copies}, Pool {stat
  combines, xh/u tail blocks, sig, dw scale}, PE {matmuls, transposes}
"""

import numpy as np
from contextlib import ExitStack

import concourse.bass as bass
from concourse import bacc
import concourse.tile as tile
from concourse import mybir
from concourse.bass_utils import run_bass_kernel_spmd

B, D, H, L = 131072, 128, 128, 4
NCORES = 8
GBLK = 4
NS = 4
STAG = 0
LN_EPS = 1e-5
FP16 = mybir.dt.float16
F32 = mybir.dt.float32
AF = mybir.ActivationFunctionType
OP = mybir.AluOpType

# act_info.json set 6 = natural_log_exp_and_others: {Exp, Ln, Square, Copy, ...}
ACT_SET_NLE = 6


def _emit(nc, R):
    G = GBLK
    NG = R // (G * 128)
    assert NG * G * 128 == R

    pt_in = nc.dram_tensor("pt", [NG, 128, G, 128], FP16, kind="ExternalInput")
    wt_in = nc.dram_tensor("wt", [NG, 128, G, 128], FP16, kind="ExternalInput")
    # fwd moving weights [K, 5, N]: {W_in.T, Wg0.T, Wg1.T, Wg2.T, M2.T}
    wf_in = nc.dram_tensor("wf", [128, 5, 128], FP16, kind="ExternalInput")
    # bwd moving weights [K, 5, N]: {M2c, Wc2, Wc1, Wc0, W_in}
    wb_in = nc.dram_tensor("wb", [128, 5, 128], FP16, kind="ExternalInput")
    cb_in = nc.dram_tensor("cb", [1, 5, 128], FP16, kind="ExternalInput")
    ones_in = nc.dram_tensor("ones1", [1, 128], FP16, kind="ExternalInput")
    id_in = nc.dram_tensor("ident", [128, 128], FP16, kind="ExternalInput")
    dp_out = nc.dram_tensor("dp", [NG, 128, G, 128], FP16, kind="ExternalOutput")
    dw_out = nc.dram_tensor("dw", [NG, 128, G, 128], FP16, kind="ExternalOutput")

    # pre-place the combined Exp+Ln act table so the fixpoint pass inserts
    # no further LoadActFuncSet
    pre = mybir.InstLoadActFuncSet(
        name=nc.get_next_instruction_name(), ins=[], outs=[],
        act_func_set_id=ACT_SET_NLE,
    )
    nc.scalar.add_instruction(pre)

    with tile.TileContext(nc) as tc, ExitStack() as ctx:
        consts = ctx.enter_context(tc.tile_pool(name="consts", bufs=1))
        io = ctx.enter_context(tc.tile_pool(name="io", bufs=3))
        work = ctx.enter_context(tc.tile_pool(name="work", bufs=2))
        saves = ctx.enter_context(tc.tile_pool(name="saves", bufs=2))
        stats = ctx.enter_context(tc.tile_pool(name="stats", bufs=2))
        zpf = [
            ctx.enter_context(tc.tile_pool(name=f"zpf{s}", bufs=1, space="PSUM"))
            for s in range(NS)
        ]
        zpb = [
            ctx.enter_context(tc.tile_pool(name=f"zpb{s}", bufs=1, space="PSUM"))
            for s in range(NS)
        ]

        wfs = consts.tile([128, 5, 128], FP16, tag="wfs")
        wbs = consts.tile([128, 5, 128], FP16, tag="wbs")
        cbs = consts.tile([1, 5, 128], FP16, tag="cbs")
        ones1 = consts.tile([1, 128], FP16, tag="ones1")
        ident = consts.tile([128, 128], FP16, tag="ident")
        epsb = consts.tile([128, 1], F32, tag="epsb")
        zerob = consts.tile([128, 1], F32, tag="zerob")
        oneb = consts.tile([128, 1], F32, tag="oneb")
        nc.vector.memset(epsb, LN_EPS)
        nc.vector.memset(zerob, 0.0)
        nc.vector.memset(oneb, 1.0)
        with tc.high_priority(offset=300000):
            nc.sync.dma_start(out=cbs[:], in_=cb_in[:, :, :])
            nc.sync.dma_start(out=ones1[:], in_=ones_in[:, :])
            nc.sync.dma_start(out=wfs[:], in_=wf_in[:, :, :])
        nc.gpsimd.dma_start(out=wbs[:], in_=wb_in[:, :, :])
        nc.gpsimd.dma_start(out=ident[:], in_=id_in[:, :])

        def mm_layer(zp, Xst, widx, with_bias):
            for b in range(G):
                if with_bias:
                    nc.tensor.matmul(
                        zp[:, b, :], ones1[:, :], cbs[:, widx, :],
                        start=True, stop=False,
                    )
                nc.tensor.matmul(
                    zp[:, b, :], Xst[:, b, :], wfs[:, widx, :] if with_bias
                    else wbs[:, widx, :],
                    start=not with_bias, stop=True,
                )

        def transpose_copy(src_h, tag, pool, ptag, copy_engine):
            # transposes live in the stream's own PSUM bank (same 2048-byte
            # tag as the matmul tile, next pool generation)
            tp = pool.tile([128, 2 * G, 128], FP16, tag=ptag)
            for b in range(G):
                nc.tensor.transpose(tp[:, b, :], src_h[:, b, :], ident[:, :])
            dst = work.tile([128, G, 128], FP16, tag=tag)
            if copy_engine is nc.scalar:
                nc.scalar.copy(dst, tp[:, 0:G, :])
            else:
                copy_engine.tensor_copy(dst, tp[:, 0:G, :])
            return dst

        def transpose_mul(srcA, srcB, tag, pool, ptag):
            # dst = (srcA^T) * (srcB^T) per block; both transposed into one
            # [128, 2G, 128] fp16 generation of the stream's PSUM bank, and
            # the elementwise multiply rides the PSUM drain (2x_1p, same
            # cost as a plain copy) -- deletes the dz TensorTensor.
            tp = pool.tile([128, 2 * G, 128], FP16, tag=ptag)
            for b in range(G):
                nc.tensor.transpose(tp[:, b, :], srcA[:, b, :], ident[:, :])
            for b in range(G):
                nc.tensor.transpose(tp[:, G + b, :], srcB[:, b, :], ident[:, :])
            dst = work.tile([128, G, 128], FP16, tag=tag)
            nc.vector.tensor_tensor(out=dst, in0=tp[:, 0:G, :],
                                    in1=tp[:, G:2 * G, :], op=OP.mult)
            return dst

        def emit_fwd(g, s, sv):
            """Forward for group g on stream s; sv collects per-layer saves."""
            # first chunk only: bias stream 0's input to the front of the SP
            # queue (it gates the pipeline start; observed order was reversed)
            off = 200000 + ((NS - 1 - s) * 2000 if g < NS else 0)
            with tc.high_priority(offset=off):
                Xst = io.tile([128, G, 128], FP16, tag=f"ptin{s}")
                nc.sync.dma_start(out=Xst, in_=pt_in[g])
            yield

            for i in range(L):
                zp = zpf[s].tile([128, G, 128], F32, tag=f"zpf{s}")
                mm_layer(zp, Xst, i, True)

                nc.scalar.activation(zp, zp, AF.Exp, bias=zerob[:, :])
                A = work.tile([128, G, 128], FP16, tag=f"A{s}")
                nc.scalar.activation(A, zp, AF.Ln, bias=oneb[:, :])
                yield

                # mean/var per block via bn_stats (even/odd split combine):
                # mu = (me+mo)/2 ; var = (Me+Mo)/128 + ((me-mo)/2)^2
                # lnv absorbs the /128: Ln(varpre/128 + eps), varpre=M+32*d^2
                bn6 = stats.tile([128, G, 6], F32, tag=f"bn6{s}")
                for b in range(G):
                    nc.vector.bn_stats(out=bn6[:, b, :], in_=A[:, b, :])
                me = bn6[:, :, 1:2]
                Me = bn6[:, :, 2:3]
                mo = bn6[:, :, 4:5]
                Mo = bn6[:, :, 5:6]
                mu = stats.tile([128, G, 1], F32, tag=f"mu{s}")
                d_ = stats.tile([128, G, 1], F32, tag=f"d{s}")
                dd = stats.tile([128, G, 1], F32, tag=f"dd{s}")
                t_ = stats.tile([128, G, 1], F32, tag=f"t{s}")
                M_ = stats.tile([128, G, 1], F32, tag=f"M{s}")
                vp = stats.tile([128, G, 1], F32, tag=f"vp{s}")
                nc.gpsimd.tensor_tensor(out=mu, in0=me, in1=mo, op=OP.add)
                nc.gpsimd.tensor_scalar_mul(mu, mu, 0.5)
                nc.gpsimd.tensor_tensor(out=d_, in0=me, in1=mo, op=OP.subtract)
                nc.gpsimd.tensor_tensor(out=dd, in0=d_, in1=d_, op=OP.mult)
                nc.gpsimd.tensor_scalar_mul(t_, dd, 32.0)
                nc.gpsimd.tensor_tensor(out=M_, in0=Me, in1=Mo, op=OP.add)
                nc.gpsimd.tensor_tensor(out=vp, in0=M_, in1=t_, op=OP.add)
                lnv = stats.tile([128, G, 1], F32, tag=f"lnv{s}")
                rstd = stats.tile([128, G, 1], F32, tag=f"rstd{i}{s}")
                nc.scalar.activation(lnv, vp, AF.Ln, bias=epsb[:, :],
                                     scale=1.0 / H)
                nc.scalar.activation(rstd, lnv, AF.Exp, scale=-0.5,
                                     bias=zerob[:, :])
                yield

                # xh = (A - mu) * rstd, per-block dual-scalar TS (DVE/Pool split)
                xh = saves.tile([128, G, 128], FP16, tag=f"xh{i}{s}")
                for b in range(G):
                    eng = nc.vector if b < 3 * G // 4 else nc.gpsimd
                    eng.tensor_scalar(
                        out=xh[:, b, :], in0=A[:, b, :],
                        scalar1=mu[:, b, :], scalar2=rstd[:, b, :],
                        op0=OP.subtract, op1=OP.mult)

                Xst = transpose_copy(xh, f"xstf{s}", zpf[s], f"zpf{s}",
                                     nc.vector)

                # sig = 1 - exp(-A)  (off the fwd critical path: deprioritize)
                with tc.high_priority(offset=-100000):
                    s1m = work.tile([128, G, 128], FP16, tag=f"s1m{s}")
                    nc.scalar.activation(s1m, A, AF.Exp, scale=-1.0,
                                         bias=zerob[:, :])
                    sig = saves.tile([128, G, 128], FP16, tag=f"sig{i}{s}")
                    nc.gpsimd.tensor_scalar(
                        out=sig, in0=s1m, scalar1=-1.0, scalar2=1.0,
                        op0=OP.mult, op1=OP.add)
                sv.append((xh, sig, rstd))
                yield

            zp = zpf[s].tile([128, G, 128], F32, tag=f"zpf{s}")
            mm_layer(zp, Xst, 4, True)
            with tc.high_priority(offset=5000):
                yo = io.tile([128, G, 128], FP16, tag=f"yout{s}")
                nc.scalar.copy(yo, zp)
            nc.sync.dma_start(out=dp_out[g], in_=yo)
            yield

        def emit_bwd(g, s, sv):
            with tc.high_priority(offset=200000):
                Gst = io.tile([128, G, 128], FP16, tag=f"wtin{s}")
                nc.sync.dma_start(out=Gst, in_=wt_in[g])
            yield

            F = None
            for i in range(L - 1, -1, -1):
                gp = zpb[s].tile([128, G, 128], F32, tag=f"zpb{s}")
                mm_layer(gp, Gst, 3 - i, False)
                gd = work.tile([128, G, 128], FP16, tag=f"gd{s}")
                nc.scalar.copy(gd, gp)
                yield

                xh, sig, rstd = sv[i]
                # m2 = sum_f(xh * gd) per row
                prod = work.tile([128, G, 128], FP16, tag=f"prod{s}")
                nc.vector.tensor_tensor(out=prod, in0=xh, in1=gd, op=OP.mult)
                # fold the /H into the accumulate pass: accum = sum(prod)/H
                qh = stats.tile([128, G, 1], F32, tag=f"qh{s}")
                for b in range(G):
                    nc.vector.tensor_scalar(
                        out=prod[:, b, :], in0=prod[:, b, :], scalar1=1.0 / H,
                        scalar2=None, op0=OP.mult, op1=OP.add,
                        accum_out=qh[:, b, :])
                yield
                # dz = (gd - xh*qh) * sig   (rstd folded into final dw copy)
                u = work.tile([128, G, 128], FP16, tag=f"u{s}")
                for b in range(G):
                    eng = nc.vector if b < G // 2 else nc.gpsimd
                    eng.tensor_scalar(
                        out=u[:, b, :], in0=xh[:, b, :], scalar1=qh[:, b, :],
                        scalar2=None, op0=OP.mult)
                dxn = work.tile([128, G, 128], FP16, tag=f"dxn{s}")
                nc.vector.tensor_tensor(out=dxn, in0=gd, in1=u, op=OP.subtract)
                dz = work.tile([128, G, 128], FP16, tag=f"dz{s}")
                nc.vector.tensor_tensor(out=dz, in0=dxn, in1=sig, op=OP.mult)

                if F is None:
                    F = rstd
                else:
                    Fn = stats.tile([128, G, 1], F32, tag=f"F{i}{s}")
                    nc.gpsimd.tensor_tensor(out=Fn, in0=F, in1=rstd,
                                            op=OP.mult)
                    F = Fn

                Gst = transpose_copy(dz, f"xstb{s}", zpb[s], f"zpb{s}",
                                     nc.vector)
                yield

            gp = zpb[s].tile([128, G, 128], F32, tag=f"zpb{s}")
            mm_layer(gp, Gst, 4, False)
            with tc.high_priority(offset=5000):
                dwu = work.tile([128, G, 128], FP16, tag=f"dwu{s}")
                nc.scalar.copy(dwu, gp)
            with tc.high_priority(offset=-100000):
                dwo = io.tile([128, G, 128], FP16, tag=f"dwout{s}")
                for b in range(G):
                    nc.gpsimd.tensor_scalar(
                        out=dwo[:, b, :], in0=dwu[:, b, :], scalar1=F[:, b, :],
                        scalar2=None, op0=OP.mult)
                nc.sync.dma_start(out=dw_out[g], in_=dwo)
            yield

        # chunked rolling pipeline with staggered stream starts: fwd chunk k
        # overlaps bwd chunk k-1; within a chunk streams start STAG steps apart
        live = []

        def step_all(n=1):
            for _ in range(n):
                for it in list(live):
                    try:
                        next(it)
                    except StopIteration:
                        live.remove(it)

        def drive_until(targets):
            while any(t in live for t in targets):
                step_all()

        chunks = []
        g0 = 0
        while g0 < NG:
            n = min(NS, NG - g0)
            if NG - g0 - n == 1:
                n -= 1
            chunks.append((g0, n))
            g0 += n
        for g0, n in chunks:
            svs = [[] for _ in range(n)]
            fgs = []
            for s in range(n):
                fg = iter(emit_fwd(g0 + s, s, svs[s]))
                live.append(fg)
                fgs.append(fg)
                step_all(STAG)
            pending = list(range(n))
            while any(fgs[s] in live for s in range(n)):
                step_all()
                for s in list(pending):
                    if fgs[s] not in live:
                        live.append(iter(emit_bwd(g0 + s, s, svs[s])))
                        pending.remove(s)
            for s in pending:
                live.append(iter(emit_bwd(g0 + s, s, svs[s])))
        drive_until(list(live))


def _host_precompute(t, W_in, b_in, fw, fb, gamma, beta, Wl, bl, W_out, b_out):
    t = np.asarray(t, dtype=np.float32).reshape(-1)[0]
    s = np.sin(t * np.asarray(fw, np.float32) + np.asarray(fb, np.float32))
    Wl = np.asarray(Wl, np.float32)
    gamma = np.asarray(gamma, np.float32)
    beta = np.asarray(beta, np.float32)
    bl = np.asarray(bl, np.float32)
    W_in = np.asarray(W_in, np.float32)
    W_out = np.asarray(W_out, np.float32)
    b_in = np.asarray(b_in, np.float32)
    b_out = np.asarray(b_out, np.float32)

    Wg = [Wl[i] * gamma[i][None, :] for i in range(L)]
    bg = [bl[i] + Wl[i] @ beta[i] for i in range(L)]

    M2 = (W_out.astype(np.float64) @ Wg[L - 1].astype(np.float64)).astype(np.float32)
    c = np.zeros((5, 128), np.float32)
    c[0] = b_in + s[0]
    for i in range(1, L):
        c[i] = bg[i - 1] + s[i]
    c[4] = b_out + W_out @ bg[L - 1]
    WF = np.stack([W_in.T] + [Wg[i].T for i in range(L - 1)] + [M2.T], axis=0)
    Wc = [Wg[i] - Wg[i].mean(axis=1, keepdims=True) for i in range(L - 1)]
    M2n = -M2
    M2c = M2n - M2n.mean(axis=1, keepdims=True)
    WB = np.stack([M2c, Wc[2], Wc[1], Wc[0], W_in], axis=0)

    WF = np.ascontiguousarray(np.transpose(WF, (1, 0, 2))).astype(np.float16)
    WB = np.ascontiguousarray(np.transpose(WB, (1, 0, 2))).astype(np.float16)
    CB = c.astype(np.float16)[None, :, :]
    ONES = np.ones((1, 128), np.float16)
    EYE = np.eye(128, dtype=np.float16)
    return WF, WB, CB, ONES, EYE


_NC_CACHE = {}


def _get_nc(R):
    if R not in _NC_CACHE:
        nc = bacc.Bacc("TRN2")
        _emit(nc, R)
        nc.finalize()
        _NC_CACHE[R] = nc
    return _NC_CACHE[R]


def _pretranspose(x, R):
    # [R, D] row-major -> [NG, 128 feat, G, 128 rows] fp16
    NG = R // (GBLK * 128)
    x = x.reshape(NG, GBLK, 128, D).transpose(0, 3, 1, 2)
    return np.ascontiguousarray(x.astype(np.float16))


def _unshuffle(y, R):
    # [NG, 128 rows, G, 128 feat] fp16 -> [R, D] f32
    return np.ascontiguousarray(
        y.astype(np.float32).transpose(0, 2, 1, 3).reshape(R, D)
    )


def _run(p, w, consts, R, n_cores):
    WF, WB, CB, ONES, EYE = consts
    nc = _get_nc(R)
    in_maps = []
    for k in range(n_cores):
        in_maps.append(
            {
                "pt": _pretranspose(p[k * R:(k + 1) * R], R),
                "wt": _pretranspose(w[k * R:(k + 1) * R], R),
                "wf": WF,
                "wb": WB,
                "cb": CB,
                "ones1": ONES,
                "ident": EYE,
            }
        )
    res = run_bass_kernel_spmd(nc, in_maps, core_ids=list(range(n_cores)))
    dp = np.concatenate([_unshuffle(r["dp"], R) for r in res.results], axis=0)
    dw = np.concatenate([_unshuffle(r["dw"], R) for r in res.results], axis=0)
    return dp, dw


def kernel(t, p, w, W_in, b_in, fw, fb, gamma, beta, Wl, bl, W_out, b_out):
    consts = _host_precompute(
        t, W_in, b_in, fw, fb, gamma, beta, Wl, bl, W_out, b_out
    )
    p = np.asarray(p, np.float32)
    w = np.asarray(w, np.float32)
    R = p.shape[0] // NCORES
    dp, dw = _run(p, w, consts, R, NCORES)
    return dp, dw

